# revision 25
# baseline (speedup 1.0000x reference)
"""Trainium2 Bass kernel for nn_BTRLoss: grayscale morphological opening loss.

Per image: tip = MLP(grid, t) [16x16]; eroded = erosion(image, tip);
recon = dilation(eroded, tip); loss = mean((recon-image)^2) + regularizers.
One image per NeuronCore (data-parallel over the batch of 8).

Algorithm: the two 256-tap max-plus convolutions are computed in the
log/tropical-softmax domain so they become ordinary LINEAR 2D convolutions
that run on the (otherwise idle) 128x128 PE array instead of 512 serial
DVE min/max passes:

    eroded = -max_{u,v}(T - P)  ~=  -(1/b) ln( corr2d(exp(-b P), exp(b T)) )
    recon  =  max_{u,v}(T + E)  ~=   (1/b) ln( corr2d(exp(b E),  exp(b T)) )

with exp(b*eroded) = 1/S available as an exact elementwise reciprocal of the
erosion conv result S (no exp/log needed between the two convs).  b ~ 15 is
chosen per image so every fp32 exponent stays in range; the smooth-max bias
is O(ln(multiplicity)/b) per pixel and measured at ~4e-4 relative error on
the total loss (tolerance 2e-2); host-side prototype proto.py validates.

Device implementation per core:
- layout: rows chunked 10x113 (plus 15 halo rows = 128 partitions per
  chunk); corr2d = 16 PSUM-accumulated bf16 matmuls per [113,512] output
  tile with banded-Toeplitz stationary weights W_v[pin,pout] = K[pin-pout,v]
  (K = exp(b*tip), built on host).  2 convs x 10 chunks x 2 col-halves x 16
  taps = 640 matmuls of 512 moving rows ~ 140us PE.
- erosion tail: DVE reciprocal_approx_fast + bf16 cast; halo rebuild via 3
  band DMAs per chunk into a memset-to-1.0 padded buffer (exp(0)=1 borders
  reproduce the reference's zero padding).
- dilation tail: ACT Ln, DVE subtract of b*I (fp16 upload), ACT Square with
  per-partition accumulate; host sums 128 partials, divides by b^2, adds the
  closed-form regularizer terms (exact, from the host-computed tip MLP).
"""
import numpy as np

try:
    import concourse.bass as bass
except ImportError:
    import sys
    for p in ("/opt/trn_rl_repo", "/root/.axon_site/_ro/trn_rl_repo"):
        if p not in sys.path:
            sys.path.insert(0, p)
    import concourse.bass as bass

import ml_dtypes
import concourse.bacc as bacc
import concourse.tile as tile
from concourse import mybir
from concourse.bass_utils import run_bass_kernel_spmd

# ---- problem geometry (hardcoded per spec) ----
B, H, W = 8, 1024, 1024
K = 16
PB = 7                   # (K-1)//2 pad before
CH = 113                 # output rows per chunk (128 - 15 halo)
NCH = 10                 # ceil(1024/113)
XW = 1040                # padded-column buffer width (needs 1039)
HB = 512                 # column half width (PSUM bank = 512 fp32)

F32 = mybir.dt.float32
F16 = mybir.dt.float16
BF16 = mybir.dt.bfloat16

# tip grid (matches reference)
_x = np.linspace(-K / 2, K / 2, K, dtype=np.float32)
_X, _Y = np.meshgrid(_x, _x, indexing="ij")
XF = _X.reshape(-1)
YF = _Y.reshape(-1)


def _tip_mlp(t, w1, b1, w2, b2, w3, b3):
    inp = np.stack([XF, YF, np.full(K * K, t, np.float32)], axis=-1)
    h = np.tanh((inp @ w1 + b1).astype(np.float32)).astype(np.float32)
    h = np.tanh((h @ w2 + b2).astype(np.float32)).astype(np.float32)
    return ((h @ w3 + b3)[..., 0]).astype(np.float32)  # [256]


def build_nc():
    nc = bacc.Bacc("TRN2", target_bir_lowering=False)
    xe_d = nc.dram_tensor("xe", [128, NCH * XW], BF16, kind="ExternalInput")
    io_d = nc.dram_tensor("iout", [128, NCH * 1024], F16, kind="ExternalInput")
    w_d = nc.dram_tensor("wmat", [128, K * 128], BF16, kind="ExternalInput")
    out_d = nc.dram_tensor("psum", [128, 4 * NCH], F32, kind="ExternalOutput")

    LN = mybir.ActivationFunctionType.Ln
    SQ = mybir.ActivationFunctionType.Square
    sub = mybir.AluOpType.subtract

    with tile.TileContext(nc) as tc:
        with tc.tile_pool(name="sb", bufs=1) as sb, \
             tc.tile_pool(name="pp", bufs=4, space="PSUM") as pp, \
             tc.tile_pool(name="sc", bufs=2) as scp, \
             tc.tile_pool(name="ln", bufs=2) as lnp, \
             tc.tile_pool(name="df", bufs=2) as dfp:
            WT = sb.tile([128, K, 128], BF16)
            XeT = sb.tile([128, NCH, XW], BF16)
            Yi = sb.tile([128, NCH, XW], BF16)      # exp(b*eroded) w/ halos
            T0 = sb.tile([128, XW], BF16)           # top-edge dilation input
            IoT = sb.tile([128, NCH, 1024], F16)    # beta * image
            ps = sb.tile([128, 4 * NCH], F32)       # [SumL2 cols | SumLI cols]

            nc.vector.memset(ps, 0.0)
            nc.vector.memset(Yi, 1.0)               # exp(0): zero-pad borders
            nc.vector.memset(T0, 1.0)

            # --- input DMAs; W + first Xe chunks gate the matmul start, so
            # W comes in 4 pieces on two queues and the first two Xe chunks
            # in column halves ---
            for piece in range(4):
                (nc.sync, nc.gpsimd)[piece % 2].dma_start(
                    out=WT[:, 4 * piece:4 * piece + 4, :],
                    in_=w_d[:, 4 * piece * 128:(4 * piece + 4) * 128])
            for c in range(2):
                nc.scalar.dma_start(out=XeT[:, c, 0:528],
                                    in_=xe_d[:, c * XW:c * XW + 528])
                (nc.gpsimd, nc.sync)[c].dma_start(
                    out=XeT[:, c, 528:XW],
                    in_=xe_d[:, c * XW + 528:(c + 1) * XW])
            for c in range(2, NCH):
                (nc.gpsimd, nc.scalar)[c % 2].dma_start(
                    out=XeT[:, c, :], in_=xe_d[:, c * XW:(c + 1) * XW])

            # --- erosion: S = corr2d(Xe, K); Yi[0:113] = bf16(1/S).
            # The dilation chunk grid is shifted +7 rows so its contraction
            # window is exactly Yi chunk c rows 0..112 plus a 15-row fringe
            # from chunk c+1 copied into partitions 113..127 -- no whole-band
            # relayout needed. ---
            for c in range(NCH):
                nv = min(CH, H - CH * c)
                for h in range(2):
                    pt = pp.tile([128, HB], F32, name="pe")
                    for v in range(K):
                        o = HB * h + v
                        nc.tensor.matmul(pt, WT[:, v, :], XeT[:, c, o:o + HB],
                                         start=(v == 0), stop=(v == K - 1))
                    rc = scp.tile([128, HB], F32, name="rc")
                    nc.vector.reciprocal_approx_fast(
                        rc[0:nv, :], pt[0:nv, :])
                    nc.vector.tensor_scalar_add(
                        Yi[0:nv, c, PB + HB * h:PB + HB * (h + 1)],
                        rc[0:nv, :], 0.0)
                # 15-row halo fringe into the previous chunk's partitions
                # 113..127 (chunk 9 contributes 7 valid rows + 8 border ones)
                dq = (nc.gpsimd, nc.sync)[c % 2]
                if c == 0:
                    dq.dma_start(out=T0[PB:PB + 15, PB:PB + 1024],
                                 in_=Yi[0:15, 0, PB:PB + 1024])
                else:
                    dq.dma_start(out=Yi[CH:128, c - 1, PB:PB + 1024],
                                 in_=Yi[0:15, c, PB:PB + 1024])

            # Iout arrives during the erosion phase; it is only read by the
            # dilation-phase loss ops
            for c in range(NCH):
                qs = (nc.sync, nc.scalar, nc.gpsimd)
                qs[c % 3].dma_start(out=IoT[:, c, :],
                                    in_=io_d[:, c * 1024:(c + 1) * 1024])

            # --- dilation: S2 = corr2d(Epad, K); loss partials.
            # Chunk k=0 is the 7-row top edge (input T0 = 7 border rows + 15
            # eroded rows); chunks k=1..9 output rows 113(k-1)+7..+119.
            # sum((L - bI)^2) = sum(L^2) - 2*sum(L*bI) + sum(bI^2): the last
            # term is exact on the host, so ACT only runs Ln+Square and DVE
            # accumulates L*bI independently -- no cross-engine chain. ---
            mul = mybir.AluOpType.mult
            for k in range(NCH):
                nv = PB if k == 0 else CH
                for h in range(2):
                    pt2 = pp.tile([128, HB], F32, name="pd")
                    for v in range(K):
                        o = HB * h + v
                        rhs = T0[:, o:o + HB] if k == 0 \
                            else Yi[:, k - 1, o:o + HB]
                        nc.tensor.matmul(pt2, WT[:, v, :], rhs,
                                         start=(v == 0), stop=(v == K - 1))
                    lnT = lnp.tile([128, HB], F32, name="ln")
                    nc.scalar.activation(lnT[0:nv, :], pt2[0:nv, :], LN)
                    col = 2 * k + h
                    sqT = dfp.tile([128, HB], F32, name="sq")
                    nc.scalar.activation(sqT[0:nv, :], lnT[0:nv, :], SQ,
                                         accum_out=ps[0:nv, col:col + 1])
                    liT = dfp.tile([128, HB], F32, name="li")
                    nc.vector.scalar_tensor_tensor(
                        out=liT[0:nv, :], in0=lnT[0:nv, :], scalar=1.0,
                        in1=IoT[0:nv, k, HB * h:HB * (h + 1)],
                        op0=mul, op1=mul,
                        accum_out=ps[0:nv, 2 * NCH + col:2 * NCH + col + 1])

            nc.sync.dma_start(out=out_d[:, :], in_=ps)
    nc.compile()
    return nc


_NC_CACHE = {}


def _get_nc():
    if "nc" not in _NC_CACHE:
        _NC_CACHE["nc"] = build_nc()
    return _NC_CACHE["nc"]


def _choose_beta(img, bh):
    t_max = float(bh.max())
    p_min = float(img.min())
    p_max = float(img.max())
    caps = [15.0]
    if t_max - p_min > 0:
        caps.append(79.0 / (t_max - p_min))   # erosion conv overflow
    if -p_min > 0:
        caps.append(82.0 / (-p_min))          # dilation conv underflow
    if p_max > 0:
        caps.append(79.0 / p_max)             # dilation conv overflow
    return min(caps)


def _prep_image(img, bh, beta):
    """Build the three per-core upload tensors for one image."""
    T = bh.reshape(K, K)
    Khat = np.exp(beta * T).astype(np.float32)            # [16,16]

    # banded-Toeplitz weights W[p, v, q] = Khat[p-q, v] (0 <= p-q < 16)
    p = np.arange(128)[:, None]
    q = np.arange(128)[None, :]
    d = p - q
    mask = (d >= 0) & (d < K)
    Wf = np.zeros((128, 128, K), np.float32)
    Wf[mask] = Khat[d[mask], :]
    wmat = np.ascontiguousarray(
        Wf.transpose(0, 2, 1)).reshape(128, K * 128).astype(ml_dtypes.bfloat16)

    # padded exp image, chunked with 15-row overlap: [128, 10, 1040]
    full = np.zeros((CH * (NCH - 1) + 128, XW), np.float32)
    full[PB:PB + H, PB:PB + W] = img
    Xf = np.exp(-beta * full)
    idx = (CH * np.arange(NCH))[:, None] + np.arange(128)[None, :]
    xe = np.ascontiguousarray(
        Xf[idx].transpose(1, 0, 2)).reshape(128, NCH * XW).astype(
            ml_dtypes.bfloat16)

    # beta*image in dilation-chunk layout: chunk 0 is the 7-row top edge
    # (partition q = image row q), chunks k=1..9 have partition q = image
    # row 113(k-1) + 7 + q
    rows = np.zeros((CH * (NCH - 1) + 128 + PB, W), np.float32)
    rows[0:H] = beta * img
    bases = np.array([0] + [CH * kk + PB for kk in range(NCH - 1)])
    idx2 = bases[:, None] + np.arange(128)[None, :]
    iout = np.ascontiguousarray(
        rows[idx2].transpose(1, 0, 2)).reshape(128, NCH * 1024).astype(
            np.float16)
    # sum of (fp16-quantized beta*I)^2 -- exactly what the device multiplies
    # (over the 1024 unique image rows, not the duplicated chunk-halo rows)
    sum_i2 = float(((beta * img).astype(np.float16).astype(np.float64) ** 2)
                   .sum())
    return {"xe": xe, "iout": iout, "wmat": wmat}, sum_i2


def _prep_inputs(images, w1, b1, w2, b2, w3, b3, n):
    metas, in_maps = [], []
    for b in range(B):
        t = float(n * B + b)
        bh = _tip_mlp(t, w1, b1, w2, b2, w3, b3)
        img = images[b]
        beta = _choose_beta(img, bh)
        im, sum_i2 = _prep_image(img, bh, beta)
        metas.append((bh, beta, sum_i2))
        in_maps.append(im)
    return metas, in_maps


def _finish_loss(metas, results):
    losses = []
    for b in range(B):
        bh, beta, sum_i2 = metas[b]
        p = np.asarray(results[b]["psum"], np.float64)
        sum_l2 = float(p[:, 0:2 * NCH].sum())
        sum_li = float(p[:, 2 * NCH:4 * NCH].sum())
        s = sum_l2 - 2.0 * sum_li + sum_i2
        recon = s / (beta * beta) / (H * W)
        tip = bh.reshape(K, K)
        boundary = float(np.mean((bh + 100.0) ** 2))
        reg = float(np.sum(bh ** 2))
        cent = float(np.dot(np.abs(bh), XF)) ** 2 + \
            float(np.dot(np.abs(bh), YF)) ** 2
        avg = float(np.mean(bh)) ** 2
        height = float(np.mean(np.maximum(tip, 0.0) ** 2)) + \
            float(np.max(tip)) ** 2
        losses.append(recon + 0.1 * boundary + 1.0 * height
                      + 1e-4 * reg + 0.1 * avg + 1e-3 * cent)
    return np.array(np.mean(np.asarray(losses, np.float64)), dtype=np.float32)


def _run(inputs, trace=False, **kw):
    images = np.asarray(inputs["images"], np.float32)
    args = [np.asarray(inputs[k], np.float32)
            for k in ("w1", "b1", "w2", "b2", "w3", "b3")]
    n = int(np.asarray(inputs["n"]))
    metas, in_maps = _prep_inputs(images, *args, n)
    res = run_bass_kernel_spmd(_get_nc(), in_maps, core_ids=list(range(B)),
                               trace=trace, **kw)
    return _finish_loss(metas, res.results), res


def kernel(**inputs) -> np.ndarray:
    loss, _ = _run(inputs)
    return loss


# revision 26
# speedup vs baseline: 1.0004x; 1.0004x over previous
"""Trainium2 Bass kernel for nn_BTRLoss: grayscale morphological opening loss.

Per image: tip = MLP(grid, t) [16x16]; eroded = erosion(image, tip);
recon = dilation(eroded, tip); loss = mean((recon-image)^2) + regularizers.
One image per NeuronCore (data-parallel over the batch of 8).

Algorithm: the two 256-tap max-plus convolutions are computed in the
log/tropical-softmax domain so they become ordinary LINEAR 2D convolutions
that run on the (otherwise idle) 128x128 PE array instead of 512 serial
DVE min/max passes:

    eroded = -max_{u,v}(T - P)  ~=  -(1/b) ln( corr2d(exp(-b P), exp(b T)) )
    recon  =  max_{u,v}(T + E)  ~=   (1/b) ln( corr2d(exp(b E),  exp(b T)) )

with exp(b*eroded) = 1/S available as an exact elementwise reciprocal of the
erosion conv result S (no exp/log needed between the two convs).  b ~ 15 is
chosen per image so every fp32 exponent stays in range; the smooth-max bias
is O(ln(multiplicity)/b) per pixel and measured at ~4e-4 relative error on
the total loss (tolerance 2e-2); host-side prototype proto.py validates.

Device implementation per core:
- layout: rows chunked 10x113 (plus 15 halo rows = 128 partitions per
  chunk); corr2d = 16 PSUM-accumulated bf16 matmuls per [113,512] output
  tile with banded-Toeplitz stationary weights W_v[pin,pout] = K[pin-pout,v]
  (K = exp(b*tip), built on host).  2 convs x 10 chunks x 2 col-halves x 16
  taps = 640 matmuls of 512 moving rows ~ 140us PE.
- erosion tail: DVE reciprocal_approx_fast + bf16 cast; halo rebuild via 3
  band DMAs per chunk into a memset-to-1.0 padded buffer (exp(0)=1 borders
  reproduce the reference's zero padding).
- dilation tail: ACT Ln, DVE subtract of b*I (fp16 upload), ACT Square with
  per-partition accumulate; host sums 128 partials, divides by b^2, adds the
  closed-form regularizer terms (exact, from the host-computed tip MLP).
"""
import numpy as np

try:
    import concourse.bass as bass
except ImportError:
    import sys
    for p in ("/opt/trn_rl_repo", "/root/.axon_site/_ro/trn_rl_repo"):
        if p not in sys.path:
            sys.path.insert(0, p)
    import concourse.bass as bass

import ml_dtypes
import concourse.bacc as bacc
import concourse.tile as tile
from concourse import mybir
from concourse.bass_utils import run_bass_kernel_spmd

# ---- problem geometry (hardcoded per spec) ----
B, H, W = 8, 1024, 1024
K = 16
PB = 7                   # (K-1)//2 pad before
CH = 113                 # output rows per chunk (128 - 15 halo)
NCH = 10                 # ceil(1024/113)
XW = 1040                # padded-column buffer width (needs 1039)
HB = 512                 # column half width (PSUM bank = 512 fp32)

F32 = mybir.dt.float32
F16 = mybir.dt.float16
BF16 = mybir.dt.bfloat16

# tip grid (matches reference)
_x = np.linspace(-K / 2, K / 2, K, dtype=np.float32)
_X, _Y = np.meshgrid(_x, _x, indexing="ij")
XF = _X.reshape(-1)
YF = _Y.reshape(-1)


def _tip_mlp(t, w1, b1, w2, b2, w3, b3):
    inp = np.stack([XF, YF, np.full(K * K, t, np.float32)], axis=-1)
    h = np.tanh((inp @ w1 + b1).astype(np.float32)).astype(np.float32)
    h = np.tanh((h @ w2 + b2).astype(np.float32)).astype(np.float32)
    return ((h @ w3 + b3)[..., 0]).astype(np.float32)  # [256]


def build_nc():
    nc = bacc.Bacc("TRN2", target_bir_lowering=False)
    xe_d = nc.dram_tensor("xe", [128, NCH * XW], BF16, kind="ExternalInput")
    io_d = nc.dram_tensor("iout", [128, NCH * 1024], F16, kind="ExternalInput")
    w_d = nc.dram_tensor("wmat", [128, K * 128], BF16, kind="ExternalInput")
    out_d = nc.dram_tensor("psum", [128, 4 * NCH], F32, kind="ExternalOutput")

    LN = mybir.ActivationFunctionType.Ln
    SQ = mybir.ActivationFunctionType.Square
    sub = mybir.AluOpType.subtract

    with tile.TileContext(nc) as tc:
        with tc.tile_pool(name="sb", bufs=1) as sb, \
             tc.tile_pool(name="pp", bufs=4, space="PSUM") as pp, \
             tc.tile_pool(name="sc", bufs=2) as scp, \
             tc.tile_pool(name="ln", bufs=2) as lnp, \
             tc.tile_pool(name="df", bufs=2) as dfp:
            WT = sb.tile([128, K, 128], BF16)
            XeT = sb.tile([128, NCH, XW], BF16)
            Yi = sb.tile([128, NCH, XW], BF16)      # exp(b*eroded) w/ halos
            T0 = sb.tile([128, XW], BF16)           # top-edge dilation input
            IoT = sb.tile([128, NCH, 1024], F16)    # beta * image
            ps = sb.tile([128, 4 * NCH], F32)       # [SumL2 cols | SumLI cols]

            nc.vector.memset(ps, 0.0)
            nc.vector.memset(Yi, 1.0)               # exp(0): zero-pad borders
            nc.vector.memset(T0, 1.0)

            # --- input DMAs; W + first Xe chunks gate the matmul start: W
            # halves go on two dedicated queues, early Xe on the third ---
            nc.sync.dma_start(out=WT[:, 0:K // 2, :],
                              in_=w_d[:, 0:(K // 2) * 128])
            nc.gpsimd.dma_start(out=WT[:, K // 2:K, :],
                              in_=w_d[:, (K // 2) * 128:K * 128])
            for c in range(2):
                nc.scalar.dma_start(out=XeT[:, c, 0:528],
                                    in_=xe_d[:, c * XW:c * XW + 528])
                (nc.sync, nc.gpsimd)[c].dma_start(
                    out=XeT[:, c, 528:XW],
                    in_=xe_d[:, c * XW + 528:(c + 1) * XW])
            for c in range(2, NCH):
                (nc.gpsimd, nc.scalar)[c % 2].dma_start(
                    out=XeT[:, c, :], in_=xe_d[:, c * XW:(c + 1) * XW])

            # --- erosion: S = corr2d(Xe, K); Yi[0:113] = bf16(1/S).
            # The dilation chunk grid is shifted +7 rows so its contraction
            # window is exactly Yi chunk c rows 0..112 plus a 15-row fringe
            # from chunk c+1 copied into partitions 113..127 -- no whole-band
            # relayout needed. ---
            for c in range(NCH):
                nv = min(CH, H - CH * c)
                for h in range(2):
                    pt = pp.tile([128, HB], F32, name="pe")
                    for v in range(K):
                        o = HB * h + v
                        nc.tensor.matmul(pt, WT[:, v, :], XeT[:, c, o:o + HB],
                                         start=(v == 0), stop=(v == K - 1))
                    rc = scp.tile([128, HB], F32, name="rc")
                    nc.vector.reciprocal_approx_fast(
                        rc[0:nv, :], pt[0:nv, :])
                    nc.vector.tensor_scalar_add(
                        Yi[0:nv, c, PB + HB * h:PB + HB * (h + 1)],
                        rc[0:nv, :], 0.0)
                # 15-row halo fringe into the previous chunk's partitions
                # 113..127 (chunk 9 contributes 7 valid rows + 8 border ones)
                dq = (nc.gpsimd, nc.sync)[c % 2]
                if c == 0:
                    dq.dma_start(out=T0[PB:PB + 15, PB:PB + 1024],
                                 in_=Yi[0:15, 0, PB:PB + 1024])
                else:
                    dq.dma_start(out=Yi[CH:128, c - 1, PB:PB + 1024],
                                 in_=Yi[0:15, c, PB:PB + 1024])

            # Iout arrives during the erosion phase; it is only read by the
            # dilation-phase loss ops
            for c in range(NCH):
                qs = (nc.sync, nc.scalar, nc.gpsimd)
                qs[c % 3].dma_start(out=IoT[:, c, :],
                                    in_=io_d[:, c * 1024:(c + 1) * 1024])

            # --- dilation: S2 = corr2d(Epad, K); loss partials.
            # Chunk k=0 is the 7-row top edge (input T0 = 7 border rows + 15
            # eroded rows); chunks k=1..9 output rows 113(k-1)+7..+119.
            # sum((L - bI)^2) = sum(L^2) - 2*sum(L*bI) + sum(bI^2): the last
            # term is exact on the host, so ACT only runs Ln+Square and DVE
            # accumulates L*bI independently -- no cross-engine chain. ---
            mul = mybir.AluOpType.mult
            for k in range(NCH):
                nv = PB if k == 0 else CH
                for h in range(2):
                    pt2 = pp.tile([128, HB], F32, name="pd")
                    for v in range(K):
                        o = HB * h + v
                        rhs = T0[:, o:o + HB] if k == 0 \
                            else Yi[:, k - 1, o:o + HB]
                        nc.tensor.matmul(pt2, WT[:, v, :], rhs,
                                         start=(v == 0), stop=(v == K - 1))
                    lnT = lnp.tile([128, HB], F32, name="ln")
                    nc.scalar.activation(lnT[0:nv, :], pt2[0:nv, :], LN)
                    col = 2 * k + h
                    sqT = dfp.tile([128, HB], F32, name="sq")
                    nc.scalar.activation(sqT[0:nv, :], lnT[0:nv, :], SQ,
                                         accum_out=ps[0:nv, col:col + 1])
                    liT = dfp.tile([128, HB], F32, name="li")
                    nc.vector.scalar_tensor_tensor(
                        out=liT[0:nv, :], in0=lnT[0:nv, :], scalar=1.0,
                        in1=IoT[0:nv, k, HB * h:HB * (h + 1)],
                        op0=mul, op1=mul,
                        accum_out=ps[0:nv, 2 * NCH + col:2 * NCH + col + 1])

            nc.sync.dma_start(out=out_d[:, :], in_=ps)
    nc.compile()
    return nc


_NC_CACHE = {}


def _get_nc():
    if "nc" not in _NC_CACHE:
        _NC_CACHE["nc"] = build_nc()
    return _NC_CACHE["nc"]


def _choose_beta(img, bh):
    t_max = float(bh.max())
    p_min = float(img.min())
    p_max = float(img.max())
    caps = [15.0]
    if t_max - p_min > 0:
        caps.append(79.0 / (t_max - p_min))   # erosion conv overflow
    if -p_min > 0:
        caps.append(82.0 / (-p_min))          # dilation conv underflow
    if p_max > 0:
        caps.append(79.0 / p_max)             # dilation conv overflow
    return min(caps)


def _prep_image(img, bh, beta):
    """Build the three per-core upload tensors for one image."""
    T = bh.reshape(K, K)
    Khat = np.exp(beta * T).astype(np.float32)            # [16,16]

    # banded-Toeplitz weights W[p, v, q] = Khat[p-q, v] (0 <= p-q < 16)
    p = np.arange(128)[:, None]
    q = np.arange(128)[None, :]
    d = p - q
    mask = (d >= 0) & (d < K)
    Wf = np.zeros((128, 128, K), np.float32)
    Wf[mask] = Khat[d[mask], :]
    wmat = np.ascontiguousarray(
        Wf.transpose(0, 2, 1)).reshape(128, K * 128).astype(ml_dtypes.bfloat16)

    # padded exp image, chunked with 15-row overlap: [128, 10, 1040]
    full = np.zeros((CH * (NCH - 1) + 128, XW), np.float32)
    full[PB:PB + H, PB:PB + W] = img
    Xf = np.exp(-beta * full)
    idx = (CH * np.arange(NCH))[:, None] + np.arange(128)[None, :]
    xe = np.ascontiguousarray(
        Xf[idx].transpose(1, 0, 2)).reshape(128, NCH * XW).astype(
            ml_dtypes.bfloat16)

    # beta*image in dilation-chunk layout: chunk 0 is the 7-row top edge
    # (partition q = image row q), chunks k=1..9 have partition q = image
    # row 113(k-1) + 7 + q
    rows = np.zeros((CH * (NCH - 1) + 128 + PB, W), np.float32)
    rows[0:H] = beta * img
    bases = np.array([0] + [CH * kk + PB for kk in range(NCH - 1)])
    idx2 = bases[:, None] + np.arange(128)[None, :]
    iout = np.ascontiguousarray(
        rows[idx2].transpose(1, 0, 2)).reshape(128, NCH * 1024).astype(
            np.float16)
    # sum of (fp16-quantized beta*I)^2 -- exactly what the device multiplies
    # (over the 1024 unique image rows, not the duplicated chunk-halo rows)
    sum_i2 = float(((beta * img).astype(np.float16).astype(np.float64) ** 2)
                   .sum())
    return {"xe": xe, "iout": iout, "wmat": wmat}, sum_i2


def _prep_inputs(images, w1, b1, w2, b2, w3, b3, n):
    metas, in_maps = [], []
    for b in range(B):
        t = float(n * B + b)
        bh = _tip_mlp(t, w1, b1, w2, b2, w3, b3)
        img = images[b]
        beta = _choose_beta(img, bh)
        im, sum_i2 = _prep_image(img, bh, beta)
        metas.append((bh, beta, sum_i2))
        in_maps.append(im)
    return metas, in_maps


def _finish_loss(metas, results):
    losses = []
    for b in range(B):
        bh, beta, sum_i2 = metas[b]
        p = np.asarray(results[b]["psum"], np.float64)
        sum_l2 = float(p[:, 0:2 * NCH].sum())
        sum_li = float(p[:, 2 * NCH:4 * NCH].sum())
        s = sum_l2 - 2.0 * sum_li + sum_i2
        recon = s / (beta * beta) / (H * W)
        tip = bh.reshape(K, K)
        boundary = float(np.mean((bh + 100.0) ** 2))
        reg = float(np.sum(bh ** 2))
        cent = float(np.dot(np.abs(bh), XF)) ** 2 + \
            float(np.dot(np.abs(bh), YF)) ** 2
        avg = float(np.mean(bh)) ** 2
        height = float(np.mean(np.maximum(tip, 0.0) ** 2)) + \
            float(np.max(tip)) ** 2
        losses.append(recon + 0.1 * boundary + 1.0 * height
                      + 1e-4 * reg + 0.1 * avg + 1e-3 * cent)
    return np.array(np.mean(np.asarray(losses, np.float64)), dtype=np.float32)


def _run(inputs, trace=False, **kw):
    images = np.asarray(inputs["images"], np.float32)
    args = [np.asarray(inputs[k], np.float32)
            for k in ("w1", "b1", "w2", "b2", "w3", "b3")]
    n = int(np.asarray(inputs["n"]))
    metas, in_maps = _prep_inputs(images, *args, n)
    res = run_bass_kernel_spmd(_get_nc(), in_maps, core_ids=list(range(B)),
                               trace=trace, **kw)
    return _finish_loss(metas, res.results), res


def kernel(**inputs) -> np.ndarray:
    loss, _ = _run(inputs)
    return loss


# revision 28
# speedup vs baseline: 1.0057x; 1.0053x over previous
"""Trainium2 Bass kernel for nn_BTRLoss: grayscale morphological opening loss.

Per image: tip = MLP(grid, t) [16x16]; eroded = erosion(image, tip);
recon = dilation(eroded, tip); loss = mean((recon-image)^2) + regularizers.
One image per NeuronCore (data-parallel over the batch of 8).

Algorithm: the two 256-tap max-plus convolutions are computed in the
log/tropical-softmax domain so they become ordinary LINEAR 2D convolutions
that run on the (otherwise idle) 128x128 PE array instead of 512 serial
DVE min/max passes:

    eroded = -max_{u,v}(T - P)  ~=  -(1/b) ln( corr2d(exp(-b P), exp(b T)) )
    recon  =  max_{u,v}(T + E)  ~=   (1/b) ln( corr2d(exp(b E),  exp(b T)) )

with exp(b*eroded) = 1/S available as an exact elementwise reciprocal of the
erosion conv result S (no exp/log needed between the two convs).  b ~ 15 is
chosen per image so every fp32 exponent stays in range; the smooth-max bias
is O(ln(multiplicity)/b) per pixel and measured at ~4e-4 relative error on
the total loss (tolerance 2e-2); host-side prototype proto.py validates.

Device implementation per core:
- layout: rows chunked 10x113 (plus 15 halo rows = 128 partitions per
  chunk); corr2d = 16 PSUM-accumulated bf16 matmuls per [113,512] output
  tile with banded-Toeplitz stationary weights W_v[pin,pout] = K[pin-pout,v]
  (K = exp(b*tip), built on host).  2 convs x 10 chunks x 2 col-halves x 16
  taps = 640 matmuls of 512 moving rows ~ 140us PE (the roofline; every
  other engine hides under it).
- erosion tail: DVE reciprocal_approx_fast + bf16 cast into the dilation
  input tile.  The dilation chunk grid is shifted +7 rows so its 128-row
  contraction window is exactly one erosion chunk's 113 output rows plus a
  15-row DMA fringe from the next chunk (partitions 113..127); a 7-row
  top-edge chunk covers image rows 0..6.  Memset-to-1.0 borders (exp(0)=1)
  reproduce the reference's zero padding.  This keeps halo traffic at
  ~0.3MB and all compute partition-0 aligned.
- dilation tail: ACT Ln then, via sum((L-bI)^2) = sum(L^2) - 2*sum(L*bI)
  + sum(bI^2), ACT Square-accumulate and an independent DVE
  scalar_tensor_tensor accumulate of L*bI (bI uploaded fp16); sum(bI^2) is
  exact on the host, which also adds the closed-form regularizer terms.
Measured ~159us/core on trn2 (PE ~86% busy at 2.4GHz; vs 2776us for the
direct 512-pass DVE/ACT max-plus implementation).
"""
import numpy as np

try:
    import concourse.bass as bass
except ImportError:
    import sys
    for p in ("/opt/trn_rl_repo", "/root/.axon_site/_ro/trn_rl_repo"):
        if p not in sys.path:
            sys.path.insert(0, p)
    import concourse.bass as bass

import ml_dtypes
import concourse.bacc as bacc
import concourse.tile as tile
from concourse import mybir
from concourse.bass_utils import run_bass_kernel_spmd

# ---- problem geometry (hardcoded per spec) ----
B, H, W = 8, 1024, 1024
K = 16
PB = 7                   # (K-1)//2 pad before
CH = 113                 # output rows per chunk (128 - 15 halo)
NCH = 10                 # ceil(1024/113)
XW = 1040                # padded-column buffer width (needs 1039)
HB = 512                 # column half width (PSUM bank = 512 fp32)

F32 = mybir.dt.float32
F16 = mybir.dt.float16
BF16 = mybir.dt.bfloat16

# tip grid (matches reference)
_x = np.linspace(-K / 2, K / 2, K, dtype=np.float32)
_X, _Y = np.meshgrid(_x, _x, indexing="ij")
XF = _X.reshape(-1)
YF = _Y.reshape(-1)


def _tip_mlp(t, w1, b1, w2, b2, w3, b3):
    inp = np.stack([XF, YF, np.full(K * K, t, np.float32)], axis=-1)
    h = np.tanh((inp @ w1 + b1).astype(np.float32)).astype(np.float32)
    h = np.tanh((h @ w2 + b2).astype(np.float32)).astype(np.float32)
    return ((h @ w3 + b3)[..., 0]).astype(np.float32)  # [256]


def build_nc():
    nc = bacc.Bacc("TRN2", target_bir_lowering=False)
    xe_d = nc.dram_tensor("xe", [128, NCH * XW], BF16, kind="ExternalInput")
    io_d = nc.dram_tensor("iout", [128, NCH * 1024], F16, kind="ExternalInput")
    w_d = nc.dram_tensor("wmat", [128, K * 128], BF16, kind="ExternalInput")
    out_d = nc.dram_tensor("psum", [128, 4 * NCH], F32, kind="ExternalOutput")

    LN = mybir.ActivationFunctionType.Ln
    SQ = mybir.ActivationFunctionType.Square
    sub = mybir.AluOpType.subtract

    with tile.TileContext(nc) as tc:
        with tc.tile_pool(name="sb", bufs=1) as sb, \
             tc.tile_pool(name="pp", bufs=4, space="PSUM") as pp, \
             tc.tile_pool(name="sc", bufs=2) as scp, \
             tc.tile_pool(name="ln", bufs=2) as lnp, \
             tc.tile_pool(name="df", bufs=2) as dfp:
            WT = sb.tile([128, K, 128], BF16)
            XeT = sb.tile([128, NCH, XW], BF16)
            Yi = sb.tile([128, NCH, XW], BF16)      # exp(b*eroded) w/ halos
            T0 = sb.tile([128, XW], BF16)           # top-edge dilation input
            IoT = sb.tile([128, NCH, 1024], F16)    # beta * image
            ps = sb.tile([128, 4 * NCH], F32)       # [SumL2 cols | SumLI cols]

            nc.vector.memset(ps, 0.0)
            nc.vector.memset(Yi, 1.0)               # exp(0): zero-pad borders
            nc.vector.memset(T0, 1.0)

            # --- input DMAs; W + first Xe chunks gate the matmul start, so
            # W is split and the first two Xe chunks come in column halves ---
            nc.sync.dma_start(out=WT[:, 0:K // 2, :],
                              in_=w_d[:, 0:(K // 2) * 128])
            nc.sync.dma_start(out=WT[:, K // 2:K, :],
                              in_=w_d[:, (K // 2) * 128:K * 128])
            for c in range(2):
                nc.gpsimd.dma_start(out=XeT[:, c, 0:528],
                                    in_=xe_d[:, c * XW:c * XW + 528])
                nc.scalar.dma_start(out=XeT[:, c, 528:XW],
                                    in_=xe_d[:, c * XW + 528:(c + 1) * XW])
            for c in range(2, NCH):
                (nc.gpsimd, nc.scalar)[c % 2].dma_start(
                    out=XeT[:, c, :], in_=xe_d[:, c * XW:(c + 1) * XW])

            # --- erosion: S = corr2d(Xe, K); Yi[0:113] = bf16(1/S).
            # The dilation chunk grid is shifted +7 rows so its contraction
            # window is exactly Yi chunk c rows 0..112 plus a 15-row fringe
            # from chunk c+1 copied into partitions 113..127 -- no whole-band
            # relayout needed. ---
            for c in range(NCH):
                nv = min(CH, H - CH * c)
                for h in range(2):
                    pt = pp.tile([128, HB], F32, name="pe")
                    for v in range(K):
                        o = HB * h + v
                        nc.tensor.matmul(pt, WT[:, v, :], XeT[:, c, o:o + HB],
                                         start=(v == 0), stop=(v == K - 1))
                    rc = scp.tile([128, HB], F32, name="rc")
                    nc.vector.reciprocal_approx_fast(
                        rc[0:nv, :], pt[0:nv, :])
                    nc.vector.tensor_scalar_add(
                        Yi[0:nv, c, PB + HB * h:PB + HB * (h + 1)],
                        rc[0:nv, :], 0.0)
                # 15-row halo fringe into the previous chunk's partitions
                # 113..127 (chunk 9 contributes 7 valid rows + 8 border ones)
                dq = (nc.gpsimd, nc.sync)[c % 2]
                if c == 0:
                    dq.dma_start(out=T0[PB:PB + 15, PB:PB + 1024],
                                 in_=Yi[0:15, 0, PB:PB + 1024])
                else:
                    dq.dma_start(out=Yi[CH:128, c - 1, PB:PB + 1024],
                                 in_=Yi[0:15, c, PB:PB + 1024])

            # Iout arrives during the erosion phase; it is only read by the
            # dilation-phase loss ops
            for c in range(NCH):
                qs = (nc.sync, nc.scalar, nc.gpsimd)
                qs[c % 3].dma_start(out=IoT[:, c, :],
                                    in_=io_d[:, c * 1024:(c + 1) * 1024])

            # --- dilation: S2 = corr2d(Epad, K); loss partials.
            # Chunk k=0 is the 7-row top edge (input T0 = 7 border rows + 15
            # eroded rows); chunks k=1..9 output rows 113(k-1)+7..+119.
            # sum((L - bI)^2) = sum(L^2) - 2*sum(L*bI) + sum(bI^2): the last
            # term is exact on the host, so ACT only runs Ln+Square and DVE
            # accumulates L*bI independently -- no cross-engine chain. ---
            mul = mybir.AluOpType.mult
            for k in range(NCH):
                nv = PB if k == 0 else CH
                for h in range(2):
                    pt2 = pp.tile([128, HB], F32, name="pd")
                    for v in range(K):
                        o = HB * h + v
                        rhs = T0[:, o:o + HB] if k == 0 \
                            else Yi[:, k - 1, o:o + HB]
                        nc.tensor.matmul(pt2, WT[:, v, :], rhs,
                                         start=(v == 0), stop=(v == K - 1))
                    lnT = lnp.tile([128, HB], F32, name="ln")
                    nc.scalar.activation(lnT[0:nv, :], pt2[0:nv, :], LN)
                    col = 2 * k + h
                    sqT = dfp.tile([128, HB], F32, name="sq")
                    nc.scalar.activation(sqT[0:nv, :], lnT[0:nv, :], SQ,
                                         accum_out=ps[0:nv, col:col + 1])
                    liT = dfp.tile([128, HB], F32, name="li")
                    nc.vector.scalar_tensor_tensor(
                        out=liT[0:nv, :], in0=lnT[0:nv, :], scalar=1.0,
                        in1=IoT[0:nv, k, HB * h:HB * (h + 1)],
                        op0=mul, op1=mul,
                        accum_out=ps[0:nv, 2 * NCH + col:2 * NCH + col + 1])

            nc.sync.dma_start(out=out_d[:, :], in_=ps)
    nc.compile()
    return nc


_NC_CACHE = {}


def _get_nc():
    if "nc" not in _NC_CACHE:
        _NC_CACHE["nc"] = build_nc()
    return _NC_CACHE["nc"]


def _choose_beta(img, bh):
    t_max = float(bh.max())
    p_min = float(img.min())
    p_max = float(img.max())
    caps = [15.0]
    if t_max - p_min > 0:
        caps.append(79.0 / (t_max - p_min))   # erosion conv overflow
    if -p_min > 0:
        caps.append(82.0 / (-p_min))          # dilation conv underflow
    if p_max > 0:
        caps.append(79.0 / p_max)             # dilation conv overflow
    return min(caps)


def _prep_image(img, bh, beta):
    """Build the three per-core upload tensors for one image."""
    T = bh.reshape(K, K)
    Khat = np.exp(beta * T).astype(np.float32)            # [16,16]

    # banded-Toeplitz weights W[p, v, q] = Khat[p-q, v] (0 <= p-q < 16)
    p = np.arange(128)[:, None]
    q = np.arange(128)[None, :]
    d = p - q
    mask = (d >= 0) & (d < K)
    Wf = np.zeros((128, 128, K), np.float32)
    Wf[mask] = Khat[d[mask], :]
    wmat = np.ascontiguousarray(
        Wf.transpose(0, 2, 1)).reshape(128, K * 128).astype(ml_dtypes.bfloat16)

    # padded exp image, chunked with 15-row overlap: [128, 10, 1040]
    full = np.zeros((CH * (NCH - 1) + 128, XW), np.float32)
    full[PB:PB + H, PB:PB + W] = img
    Xf = np.exp(-beta * full)
    idx = (CH * np.arange(NCH))[:, None] + np.arange(128)[None, :]
    xe = np.ascontiguousarray(
        Xf[idx].transpose(1, 0, 2)).reshape(128, NCH * XW).astype(
            ml_dtypes.bfloat16)

    # beta*image in dilation-chunk layout: chunk 0 is the 7-row top edge
    # (partition q = image row q), chunks k=1..9 have partition q = image
    # row 113(k-1) + 7 + q
    rows = np.zeros((CH * (NCH - 1) + 128 + PB, W), np.float32)
    rows[0:H] = beta * img
    bases = np.array([0] + [CH * kk + PB for kk in range(NCH - 1)])
    idx2 = bases[:, None] + np.arange(128)[None, :]
    iout = np.ascontiguousarray(
        rows[idx2].transpose(1, 0, 2)).reshape(128, NCH * 1024).astype(
            np.float16)
    # sum of (fp16-quantized beta*I)^2 -- exactly what the device multiplies
    # (over the 1024 unique image rows, not the duplicated chunk-halo rows)
    sum_i2 = float(((beta * img).astype(np.float16).astype(np.float64) ** 2)
                   .sum())
    return {"xe": xe, "iout": iout, "wmat": wmat}, sum_i2


def _prep_inputs(images, w1, b1, w2, b2, w3, b3, n):
    metas, in_maps = [], []
    for b in range(B):
        t = float(n * B + b)
        bh = _tip_mlp(t, w1, b1, w2, b2, w3, b3)
        img = images[b]
        beta = _choose_beta(img, bh)
        im, sum_i2 = _prep_image(img, bh, beta)
        metas.append((bh, beta, sum_i2))
        in_maps.append(im)
    return metas, in_maps


def _finish_loss(metas, results):
    losses = []
    for b in range(B):
        bh, beta, sum_i2 = metas[b]
        p = np.asarray(results[b]["psum"], np.float64)
        sum_l2 = float(p[:, 0:2 * NCH].sum())
        sum_li = float(p[:, 2 * NCH:4 * NCH].sum())
        s = sum_l2 - 2.0 * sum_li + sum_i2
        recon = s / (beta * beta) / (H * W)
        tip = bh.reshape(K, K)
        boundary = float(np.mean((bh + 100.0) ** 2))
        reg = float(np.sum(bh ** 2))
        cent = float(np.dot(np.abs(bh), XF)) ** 2 + \
            float(np.dot(np.abs(bh), YF)) ** 2
        avg = float(np.mean(bh)) ** 2
        height = float(np.mean(np.maximum(tip, 0.0) ** 2)) + \
            float(np.max(tip)) ** 2
        losses.append(recon + 0.1 * boundary + 1.0 * height
                      + 1e-4 * reg + 0.1 * avg + 1e-3 * cent)
    return np.array(np.mean(np.asarray(losses, np.float64)), dtype=np.float32)


def _run(inputs, trace=False, **kw):
    images = np.asarray(inputs["images"], np.float32)
    args = [np.asarray(inputs[k], np.float32)
            for k in ("w1", "b1", "w2", "b2", "w3", "b3")]
    n = int(np.asarray(inputs["n"]))
    metas, in_maps = _prep_inputs(images, *args, n)
    res = run_bass_kernel_spmd(_get_nc(), in_maps, core_ids=list(range(B)),
                               trace=trace, **kw)
    return _finish_loss(metas, res.results), res


def kernel(**inputs) -> np.ndarray:
    loss, _ = _run(inputs)
    return loss


# revision 46
# speedup vs baseline: 1.1001x; 1.0939x over previous
"""Trainium2 Bass kernel for nn_BTRLoss: grayscale morphological opening loss.

Per image: tip = MLP(grid, t) [16x16]; eroded = erosion(image, tip);
recon = dilation(eroded, tip); loss = mean((recon-image)^2) + regularizers.
One image per NeuronCore (data-parallel over the batch of 8).

Algorithm: the two 256-tap max-plus convolutions are computed in the
log/tropical-softmax domain so they become ordinary LINEAR 2D convolutions
that run on the (otherwise idle) 128x128 PE array instead of 512 serial
DVE min/max passes:

    eroded = -max_{u,v}(T - P)  ~=  -(1/b) ln( corr2d(exp(-b P), exp(b T)) )
    recon  =  max_{u,v}(T + E)  ~=   (1/b) ln( corr2d(exp(b E),  exp(b T)) )

with exp(b*eroded) = 1/S available as an exact elementwise reciprocal of the
erosion conv result S (no exp/log needed between the two convs).  b ~ 15 is
chosen per image so every fp32 exponent stays in range; the smooth-max bias
is O(ln(multiplicity)/b) per pixel and measured at ~4e-4 relative error on
the total loss (tolerance 2e-2); host-side prototype proto.py validates.

Device implementation per core:
- layout: rows in 9 chunks (8x113 + one 120-row boundary chunk per conv;
  halo rows fill the 128 partitions); corr2d = 16 PSUM-accumulated bf16
  matmuls per [<=120,512] output tile with banded-Toeplitz stationary
  weights W_v[pin,pout] = K[pin-pout,v] (K = exp(b*tip), built on host).
  2 convs x 9 chunks x 2 col-halves x 16 taps = 576 matmuls of 512 moving
  rows ~ 123us PE (the roofline; every other engine hides under it).
- boundary chunks are absorbed exactly: conv taps that fall outside a
  128-row contraction window only ever hit zero-pad image rows, where the
  exp-domain input is exactly 1, so their contribution is a per-output-row
  CONSTANT (partial row-sums of K) added on DVE before the reciprocal/Ln.
  The erosion's last chunk emits 120 rows this way; the dilation's first
  chunk uses a +7-shifted band W2 plus the top-pad constant to cover image
  rows 0..119.
- erosion tail: DVE reciprocal_approx_fast + bf16 cast straight into the
  dilation input tile (the dilation chunk grid is aligned so one erosion
  chunk's output IS one dilation contraction window, plus a 15-row DMA
  fringe into partitions 113..127).  Memset-to-1.0 borders (exp(0)=1)
  reproduce the reference's zero padding; halo traffic is ~0.25MB.
- dilation tail: ACT Ln then, via sum((L-bI)^2) = sum(L^2) - 2*sum(L*bI)
  + sum(bI^2), ACT Square-accumulate and an independent DVE
  scalar_tensor_tensor accumulate of L*bI (bI uploaded fp16); sum(bI^2) is
  exact on the host, which also adds the closed-form regularizer terms.
The erosion weights are uploaded as fp8e4m3 scaled by FP8C=256 (a power of
two): they gate the PE start, and halving those bytes shortens the DMA-bound
head.  The scale cancels exactly -- recip makes the dilation input 1/FP8C
smaller uniformly (borders memset to 1/FP8C), and the resulting -ln(FP8C)
shift of Ln(S2) is folded into the uploaded beta*I term.  Dilation weights
stay bf16 (off the critical path).  fp8 quantization moved the total loss
by 8e-6 relative (3.446e-4 -> 3.522e-4 vs the reference).

Measured ~144us/core on trn2 (PE ~89% busy at full 2.4GHz; vs 2776us for
the direct 512-pass DVE/ACT max-plus implementation = 19.3x).
"""
import numpy as np

try:
    import concourse.bass as bass
except ImportError:
    import sys
    for p in ("/opt/trn_rl_repo", "/root/.axon_site/_ro/trn_rl_repo"):
        if p not in sys.path:
            sys.path.insert(0, p)
    import concourse.bass as bass

import ml_dtypes
import concourse.bacc as bacc
import concourse.tile as tile
from concourse import mybir
from concourse.bass_utils import run_bass_kernel_spmd

# ---- problem geometry (hardcoded per spec) ----
B, H, W = 8, 1024, 1024
K = 16
PB = 7                   # (K-1)//2 pad before
CH = 113                 # output rows per chunk (128 - 15 halo)
NCH = 9                  # chunks per conv (boundary chunks absorbed via
                         # constant-pad correction, see build_nc)
XW = 1040                # padded-column buffer width (needs 1039)
HB = 512                 # column half width (PSUM bank = 512 fp32)

F32 = mybir.dt.float32
F16 = mybir.dt.float16
BF16 = mybir.dt.bfloat16

# tip grid (matches reference)
_x = np.linspace(-K / 2, K / 2, K, dtype=np.float32)
_X, _Y = np.meshgrid(_x, _x, indexing="ij")
XF = _X.reshape(-1)
YF = _Y.reshape(-1)


def _tip_mlp(t, w1, b1, w2, b2, w3, b3):
    inp = np.stack([XF, YF, np.full(K * K, t, np.float32)], axis=-1)
    h = np.tanh((inp @ w1 + b1).astype(np.float32)).astype(np.float32)
    h = np.tanh((h @ w2 + b2).astype(np.float32)).astype(np.float32)
    return ((h @ w3 + b3)[..., 0]).astype(np.float32)  # [256]


FP8C = 256.0             # fp8 weight scale (power of 2; ln folded into iout)
F8 = mybir.dt.float8e4


def build_nc():
    nc = bacc.Bacc("TRN2", target_bir_lowering=False)
    xe_d = nc.dram_tensor("xe", [128, NCH * XW], BF16, kind="ExternalInput")
    io_d = nc.dram_tensor("iout", [128, NCH * 1024], F16, kind="ExternalInput")
    w8_d = nc.dram_tensor("wmat8", [128, K * 128], F8, kind="ExternalInput")
    wd_d = nc.dram_tensor("wmatd", [128, K * 128], BF16, kind="ExternalInput")
    w2_d = nc.dram_tensor("wmat2", [128, K * 128], BF16, kind="ExternalInput")
    cs_d = nc.dram_tensor("cpad", [128, 2], F32, kind="ExternalInput")
    out_d = nc.dram_tensor("psum", [128, 4 * NCH], F32, kind="ExternalOutput")

    LN = mybir.ActivationFunctionType.Ln
    SQ = mybir.ActivationFunctionType.Square

    with tile.TileContext(nc) as tc:
        with tc.tile_pool(name="sb", bufs=1) as sb, \
             tc.tile_pool(name="pp", bufs=4, space="PSUM") as pp, \
             tc.tile_pool(name="sc", bufs=2) as scp, \
             tc.tile_pool(name="ln", bufs=2) as lnp, \
             tc.tile_pool(name="df", bufs=2) as dfp:
            WT8 = sb.tile([128, K, 128], F8)        # erosion band, fp8*FP8C
            WTD = sb.tile([128, K, 128], BF16)      # dilation band (unscaled)
            WT2 = sb.tile([128, K, 128], BF16)      # +7 band for dila chunk 0
            csT = sb.tile([128, 2], F32)            # pad-row constant columns
            XeT = sb.tile([128, NCH, XW], BF16)
            Yi = sb.tile([128, NCH, XW], BF16)      # exp(b*eroded)/FP8C
            IoT = sb.tile([128, NCH, 1024], F16)    # beta*image - ln(FP8C)
            ps = sb.tile([128, 4 * NCH], F32)       # [SumL2 cols | SumLI cols]

            nc.vector.memset(ps, 0.0)
            nc.vector.memset(Yi, 1.0 / FP8C)        # exp(0)/c: zero-pad rows

            # --- input DMAs; W8 + first Xe chunks gate the matmul start (W8
            # is fp8 precisely to halve those critical head bytes); the
            # dilation weights are only needed ~70us in. ---
            nc.sync.dma_start(out=csT, in_=cs_d[:, :])
            nc.sync.dma_start(out=WT8[:, 0:K // 2, :],
                              in_=w8_d[:, 0:(K // 2) * 128])
            nc.sync.dma_start(out=WT8[:, K // 2:K, :],
                              in_=w8_d[:, (K // 2) * 128:K * 128])
            for c in range(2):
                nc.gpsimd.dma_start(out=XeT[:, c, 0:528],
                                    in_=xe_d[:, c * XW:c * XW + 528])
                nc.scalar.dma_start(out=XeT[:, c, 528:XW],
                                    in_=xe_d[:, c * XW + 528:(c + 1) * XW])
            # sync drains W8 quickly (fp8, and it is the fastest queue), so
            # it carries the next two even chunks before the dilation weights
            nc.sync.dma_start(out=XeT[:, 2, :], in_=xe_d[:, 2 * XW:3 * XW])
            nc.sync.dma_start(out=XeT[:, 4, :], in_=xe_d[:, 4 * XW:5 * XW])
            for c in (3, 5, 6, 7, 8):
                (nc.gpsimd, nc.scalar)[c % 2].dma_start(
                    out=XeT[:, c, :], in_=xe_d[:, c * XW:(c + 1) * XW])
            nc.sync.dma_start(out=WT2, in_=w2_d[:, :])
            nc.sync.dma_start(out=WTD, in_=wd_d[:, :])

            # --- erosion: S = corr2d(Xe, K); Yi[0:113] = bf16(1/S).
            # The dilation chunk grid is shifted so its contraction window is
            # exactly Yi chunk c rows 0..112 plus a 15-row fringe from chunk
            # c+1 in partitions 113..127.  The last chunk (c=8) additionally
            # emits eroded rows 1017..1023 on partitions 113..119: the taps
            # falling past the 128-row window hit only zero-pad image rows
            # (exp=1), so their contribution is the per-row constant csT[:,0]
            # added before the reciprocal. ---
            for c in range(NCH):
                last = c == NCH - 1
                nv = 128 - PB - 1 if last else CH   # 120 rows on the last
                for h in range(2):
                    pt = pp.tile([128, HB], F32, name="pe")
                    for v in range(K):
                        o = HB * h + v
                        nc.tensor.matmul(pt, WT8[:, v, :], XeT[:, c, o:o + HB],
                                         start=(v == 0), stop=(v == K - 1))
                    if last:
                        nc.vector.tensor_scalar_add(pt, pt, csT[:, 0:1])
                    rc = scp.tile([128, HB], F32, name="rc")
                    nc.vector.reciprocal_approx_fast(
                        rc[0:nv, :], pt[0:nv, :])
                    nc.vector.tensor_scalar_add(
                        Yi[0:nv, c, PB + HB * h:PB + HB * (h + 1)],
                        rc[0:nv, :], 0.0)
                # 15-row halo fringe into the previous chunk's partitions
                # 113..127 (the last chunk's own fringe rows came from the
                # extended cast; rows past 1023 stay at the memset 1.0)
                if c >= 1:
                    (nc.gpsimd, nc.sync)[c % 2].dma_start(
                        out=Yi[CH:128, c - 1, PB:PB + 1024],
                        in_=Yi[0:15, c, PB:PB + 1024])

            # Iout arrives during the erosion phase; it is only read by the
            # dilation-phase loss ops
            for c in range(NCH):
                qs = (nc.sync, nc.scalar, nc.gpsimd)
                qs[c % 3].dma_start(out=IoT[:, c, :],
                                    in_=io_d[:, c * 1024:(c + 1) * 1024])

            # --- dilation: S2 = corr2d(Epad, K); loss partials.
            # Chunk k=0 outputs image rows 0..119 from Yi chunk 0 using the
            # +7-shifted band W2; the 7 top zero-pad rows contribute the
            # constant csT[:,1] (added pre-Ln).  Chunks k=1..8 output rows
            # 120+113(k-1)..+112 from Yi chunk k with the standard band.
            # sum((L - bI)^2) = sum(L^2) - 2*sum(L*bI) + sum(bI^2): the last
            # term is exact on the host, so ACT only runs Ln+Square and DVE
            # accumulates L*bI independently -- no cross-engine chain. ---
            mul = mybir.AluOpType.mult
            for k in range(NCH):
                nv = 128 - PB - 1 if k == 0 else CH
                for h in range(2):
                    pt2 = pp.tile([128, HB], F32, name="pd")
                    for v in range(K):
                        o = HB * h + v
                        wv = WT2[:, v, :] if k == 0 else WTD[:, v, :]
                        nc.tensor.matmul(pt2, wv, Yi[:, k, o:o + HB],
                                         start=(v == 0), stop=(v == K - 1))
                    if k == 0:
                        nc.vector.tensor_scalar_add(pt2, pt2, csT[:, 1:2])
                    lnT = lnp.tile([128, HB], F32, name="ln")
                    nc.scalar.activation(lnT[0:nv, :], pt2[0:nv, :], LN)
                    col = 2 * k + h
                    sqT = dfp.tile([128, HB], F32, name="sq")
                    nc.scalar.activation(sqT[0:nv, :], lnT[0:nv, :], SQ,
                                         accum_out=ps[0:nv, col:col + 1])
                    liT = dfp.tile([128, HB], F32, name="li")
                    nc.vector.scalar_tensor_tensor(
                        out=liT[0:nv, :], in0=lnT[0:nv, :], scalar=1.0,
                        in1=IoT[0:nv, k, HB * h:HB * (h + 1)],
                        op0=mul, op1=mul,
                        accum_out=ps[0:nv, 2 * NCH + col:2 * NCH + col + 1])

            nc.sync.dma_start(out=out_d[:, :], in_=ps)
    nc.compile()
    return nc


_NC_CACHE = {}


def _get_nc():
    if "nc" not in _NC_CACHE:
        _NC_CACHE["nc"] = build_nc()
    return _NC_CACHE["nc"]


def _choose_beta(img, bh):
    t_max = float(bh.max())
    p_min = float(img.min())
    p_max = float(img.max())
    caps = [15.0]
    if t_max - p_min > 0:
        # erosion conv overflow; the fp8 weight scale eats ln(FP8C) of range
        caps.append((79.0 - np.log(FP8C)) / (t_max - p_min))
    if -p_min > 0:
        caps.append(82.0 / (-p_min))          # dilation conv underflow
    if p_max > 0:
        caps.append(79.0 / p_max)             # dilation conv overflow
    return min(caps)


def _prep_image(img, bh, beta):
    """Build the per-core upload tensors for one image."""
    T = bh.reshape(K, K)
    Khat = np.exp(beta * T).astype(np.float32)            # [16,16]
    rowsum = Khat.sum(axis=1).astype(np.float64)          # [16]

    # banded-Toeplitz weights W[p, v, q] = Khat[p-q, v] (0 <= p-q < 16)
    # and the +7-shifted band W2[p, v, q] = Khat[p-q+7, v] for the first
    # dilation chunk (outputs rows 0..119)
    p = np.arange(128)[:, None]
    q = np.arange(128)[None, :]

    def band(shift):
        d = p - q + shift
        mask = (d >= 0) & (d < K)
        Wf = np.zeros((128, 128, K), np.float32)
        Wf[mask] = Khat[d[mask], :]
        return np.ascontiguousarray(Wf.transpose(0, 2, 1)).reshape(
            128, K * 128).astype(ml_dtypes.bfloat16)

    wmatd, wmat2 = band(0), band(PB)
    # erosion weights: fp8 e4m3 scaled by FP8C (clamped off the flush-to-
    # zero edge); the scale cancels through recip and shifts Ln by ln(FP8C),
    # which is folded into iout below
    Wf8 = FP8C * band(0).astype(np.float32)
    Wf8[Wf8 > 0] = np.maximum(Wf8[Wf8 > 0], 2.0 ** -9)
    wmat8 = Wf8.astype(ml_dtypes.float8_e4m3)
    # per-u scaled row sums from the quantized fp8 weights (entry [p=u, q=0]
    # of the band is Khat_q[u, v] * FP8C)
    q8 = np.asarray(wmat8.astype(np.float64)).reshape(128, K, 128)
    rowsum_s = np.array([q8[u, :, 0].sum() for u in range(K)])

    # constant-pad corrections: the conv taps that fall outside the 128-row
    # contraction window only ever hit zero-pad image rows, where the exp
    # image is exactly 1 (stored as 1/FP8C on the dilation side), so they
    # contribute a per-output-row constant.
    cpad = np.zeros((128, 2), np.float32)
    for i in range(PB):                   # erosion rows 1017..1023 (q=113+i)
        cpad[CH + i, 0] = rowsum_s[K - 1 - i:].sum()
    for qq in range(PB):                  # dilation rows 0..6
        cpad[qq, 1] = rowsum[0:PB - qq].sum() / FP8C

    # padded exp image, chunked with 15-row overlap: [128, 9, 1040]
    full = np.zeros((CH * (NCH - 1) + 128, XW), np.float32)
    full[PB:PB + H, PB:PB + W] = img
    Xf = np.exp(-beta * full)
    idx = (CH * np.arange(NCH))[:, None] + np.arange(128)[None, :]
    xe = np.ascontiguousarray(
        Xf[idx].transpose(1, 0, 2)).reshape(128, NCH * XW).astype(
            ml_dtypes.bfloat16)

    # (beta*image - ln(FP8C)) in dilation-chunk layout: the device's Ln sees
    # S2/FP8C, so L' = lnS2 - ln(FP8C) and L' - (bI - ln(FP8C)) = lnS2 - bI.
    # chunk 0 covers rows 0..119 (partition q = image row q), chunks k=1..8
    # rows 120+113(k-1)..+112
    rows = np.zeros((120 + CH * (NCH - 2) + 128, W), np.float32)
    rows[0:H] = beta * img - np.log(FP8C)
    bases = np.array([0] + [120 + CH * kk for kk in range(NCH - 1)])
    idx2 = bases[:, None] + np.arange(128)[None, :]
    iout = np.ascontiguousarray(
        rows[idx2].transpose(1, 0, 2)).reshape(128, NCH * 1024).astype(
            np.float16)
    # sum of the fp16-quantized shifted image squared -- exactly what the
    # device multiplies (over the 1024 unique image rows)
    sum_i2 = float(((beta * img - np.log(FP8C)).astype(np.float16)
                    .astype(np.float64) ** 2).sum())
    return {"xe": xe, "iout": iout, "wmat8": wmat8, "wmatd": wmatd,
            "wmat2": wmat2, "cpad": cpad}, sum_i2


def _prep_inputs(images, w1, b1, w2, b2, w3, b3, n):
    metas, in_maps = [], []
    for b in range(B):
        t = float(n * B + b)
        bh = _tip_mlp(t, w1, b1, w2, b2, w3, b3)
        img = images[b]
        beta = _choose_beta(img, bh)
        im, sum_i2 = _prep_image(img, bh, beta)
        metas.append((bh, beta, sum_i2))
        in_maps.append(im)
    return metas, in_maps


def _finish_loss(metas, results):
    losses = []
    for b in range(B):
        bh, beta, sum_i2 = metas[b]
        p = np.asarray(results[b]["psum"], np.float64)
        sum_l2 = float(p[:, 0:2 * NCH].sum())
        sum_li = float(p[:, 2 * NCH:4 * NCH].sum())
        s = sum_l2 - 2.0 * sum_li + sum_i2
        recon = s / (beta * beta) / (H * W)
        tip = bh.reshape(K, K)
        boundary = float(np.mean((bh + 100.0) ** 2))
        reg = float(np.sum(bh ** 2))
        cent = float(np.dot(np.abs(bh), XF)) ** 2 + \
            float(np.dot(np.abs(bh), YF)) ** 2
        avg = float(np.mean(bh)) ** 2
        height = float(np.mean(np.maximum(tip, 0.0) ** 2)) + \
            float(np.max(tip)) ** 2
        losses.append(recon + 0.1 * boundary + 1.0 * height
                      + 1e-4 * reg + 0.1 * avg + 1e-3 * cent)
    return np.array(np.mean(np.asarray(losses, np.float64)), dtype=np.float32)


def _run(inputs, trace=False, **kw):
    images = np.asarray(inputs["images"], np.float32)
    args = [np.asarray(inputs[k], np.float32)
            for k in ("w1", "b1", "w2", "b2", "w3", "b3")]
    n = int(np.asarray(inputs["n"]))
    metas, in_maps = _prep_inputs(images, *args, n)
    res = run_bass_kernel_spmd(_get_nc(), in_maps, core_ids=list(range(B)),
                               trace=trace, **kw)
    return _finish_loss(metas, res.results), res


def kernel(**inputs) -> np.ndarray:
    loss, _ = _run(inputs)
    return loss
